# revision 16
# baseline (speedup 1.0000x reference)
"""GCN layer kernel for Trainium2, SPMD over 8 NeuronCores.

Reference computation (all fp32):
    adj_hat = rownorm(adj + I)                      # [N, N]
    out     = adj_hat @ (X @ W) + bias              # X: [N, T, A]

Sharding: T (time) axis split across 8 cores; adj/W/bias replicated.

v2: bf16 I/O. The correctness gate is rel_err < 2e-2 and the full-bf16
datapath measures 4e-3, so X and out travel as bf16 — HBM traffic per
core drops 67MB -> 33.5MB, which was the roofline (DMA was 91% busy at
fp32). bf16 also makes every matmul 1 cyc/col at any width (no [W|W]
duplication) and enables FWL weight loads that hide LDWEIGHTS under the
previous matmul.

Per-core kernel (T_SH = 256 time steps, time blocks of tb=16):
  setup (once): load adj [m,n] fp32; r[m] = 1/(1+rowsum); fold the row
    normalization INTO the adjacency: a_hat = (adj + I) * r, then 4 PE
    transposes -> adjT_hat [n, m] in bf16. The epilogue then needs no
    per-partition scale. Load W [a,o] bf16 and a (c,t2,o)-replicated
    bias tile.
  per pair of time steps (2 t per PSUM bank, amortizes copy fixed cost):
    G1: ypt2[a, (t2 m)] = matmul(lhsT=X_t[n,a] bf16, rhs=adjT_hat[n,m])
        x2 chunks x2 t -> one [128,512] PSUM bank
    ys2 = bf16(ypt2)                          (one ACT copy per 2 t)
    G2: ops2[m, (c t2 o)] = matmul(lhsT=ys2[a, m-chunk], rhs=W[a,o])
        x2 chunks x2 t -> one [128,512] PSUM bank
    out = bf16(ops2 + bias)                   (one DVE add per 2 t)
  Each HWDGE ring tops out ~200-300GB/s, so X loads alternate between
  the sync and scalar rings and stores go on the gpsimd ring (last 4
  blocks fan out across all three to drain the tail); directions stay
  disjoint per ring so stores never head-of-line block loads; X
  prefetched 4 blocks deep.
Host: converts X/W to bf16, slices T, and upcasts the bf16 output back
to fp32.
"""

import os
import sys

import numpy as np

for _p in ("/opt/trn_rl_repo", "/root/.axon_site/_ro/trn_rl_repo"):
    if os.path.isdir(_p) and _p not in sys.path:
        sys.path.insert(0, _p)

import concourse.bass as bass
import concourse.mybir as mybir
import concourse.tile as tile
from concourse import bacc
from concourse.bass_utils import run_bass_kernel_spmd
from concourse.masks import make_identity

N_NODES = 256
N_TIMES = 2048
N_FEAT = 128
N_CORES = 8
T_SH = N_TIMES // N_CORES  # 256 time steps per core
P = 128  # partitions
NCH = N_NODES // P  # 2 node chunks

F32 = mybir.dt.float32
BF16 = mybir.dt.bfloat16


def _gcn_body(tc, out, x, adj, w, b, t_sh, tb):
    nc = tc.nc
    nblk = t_sh // tb
    ngrp = tb // 2  # 2 time steps per PSUM bank

    from contextlib import ExitStack

    with ExitStack() as ctx:
        const = ctx.enter_context(tc.tile_pool(name="const", bufs=1))

        ident = const.tile([P, P], F32)
        make_identity(nc, ident)

        w_sb = const.tile([P, N_FEAT], BF16)
        nc.gpsimd.dma_start(out=w_sb, in_=w)

        # bias replicated across partitions and duplicated (c, t4) so one
        # DVE add per 4 time steps covers a whole [c, t4, o] PSUM group
        bias_bc3 = const.tile([P, NCH * 2, N_FEAT], F32)
        bias_bcast_ap = bass.AP(
            tensor=b.tensor, offset=b.offset, ap=[[0, P], [0, NCH * 2], b.ap[0]]
        )
        nc.gpsimd.dma_start(out=bias_bc3, in_=bias_bcast_ap)
        bias_bc = bias_bc3.rearrange("p (c q) o -> p c q o", c=NCH)

        # adjT_hat[n, m] = (adj[m, n] + I) / deg[m], n on partitions, bf16
        adjT = [
            const.tile([P, N_NODES], BF16, name=f"adjT{c}", tag=f"adjT{c}")
            for c in range(NCH)
        ]

        # Main-loop SBUF pools are created BEFORE the setup scratch pool so
        # their addresses don't alias it - otherwise the first X-tile DMAs
        # inherit a WAR dependency on the whole adjacency-setup chain and the
        # DMA queue sits idle at kernel start.
        xp = ctx.enter_context(tc.tile_pool(name="xp", bufs=6))
        op = ctx.enter_context(tc.tile_pool(name="op", bufs=3))
        ysb = ctx.enter_context(tc.tile_pool(name="ysb", bufs=ngrp + 2))

        # [n, t, a] viewed as [n%128, n//128, t, a] so one 1MB DMA moves both
        # node chunks of a time block
        x4 = x.rearrange("(c n) t a -> n c t a", n=P)
        out4 = out.rearrange("(c m) t a -> m c t a", m=P)

        # Each HWDGE ring tops out around ~200GB/s, which made the single
        # load ring (17MB -> ~87us) the pacing constraint. Only sync/
        # scalar/gpsimd have HWDGE rings. Directions stay DISJOINT per
        # ring (a store descriptor waiting on its epilogue would head-of-
        # line block later loads in the same FIFO): loads alternate
        # sync/scalar, stores go to gpsimd - except the last 4 blocks'
        # stores, which fan out to sync/scalar to drain the tail in
        # parallel (by then all loads have been emitted, so no blocking).
        load_eng = [nc.sync, nc.scalar]

        def store_eng(blk):
            if blk < nblk - 4:
                return nc.gpsimd
            return [nc.sync, nc.scalar, nc.gpsimd][blk % 3]

        def load_x(blk):
            t0 = blk * tb
            xtc = xp.tile([P, NCH, tb, N_FEAT], BF16, name=f"x_{blk}", tag="x")
            load_eng[blk % 2].dma_start(out=xtc, in_=x4[:, :, t0 : t0 + tb, :])
            return xtc

        setup = ctx.enter_context(tc.tile_pool(name="setup", bufs=1))
        # the tiny adjacency loads ride the gpsimd ring, which carries no
        # bulk traffic until the first store (~25us in) - so the setup
        # chain is never queued behind megabytes of X prefetch, and the
        # X prefetch starts descriptor-gen immediately on sync/scalar
        a_sb = []
        for mc in range(NCH):
            a_t = setup.tile([P, N_NODES], F32, name=f"a{mc}", tag=f"a{mc}")
            nc.gpsimd.dma_start(out=a_t, in_=adj[mc * P : (mc + 1) * P, :])
            a_sb.append(a_t)

        PF = 4  # prefetch depth (< xp bufs)
        prefetched = [load_x(blk) for blk in range(min(PF, nblk))]

        with tc.tile_pool(name="setup_ps", bufs=1, space="PSUM") as setup_ps:
            # r[m] = 1 / (1 + sum_n adj[m, n]) off the natural [m, n] layout
            for mc in range(NCH):
                dg = setup.tile([P, 1], F32, name=f"dg{mc}", tag=f"dg{mc}")
                nc.vector.reduce_sum(dg, a_sb[mc], axis=mybir.AxisListType.X)
                nc.vector.tensor_scalar_add(dg, dg, 1.0)
                r = setup.tile([P, 1], F32, name=f"r{mc}", tag=f"r{mc}")
                nc.vector.reciprocal(r, dg)
                # fold normalization in BEFORE the transpose, while the row
                # index m is still the partition dim: (adj + I) * r
                nc.vector.tensor_scalar_mul(a_sb[mc], a_sb[mc], r)
                rdiag = setup.tile([P, P], F32, name=f"rd{mc}", tag=f"rd{mc}")
                nc.vector.tensor_scalar_mul(rdiag, ident, r)
                nc.vector.tensor_add(
                    a_sb[mc][:, mc * P : (mc + 1) * P],
                    a_sb[mc][:, mc * P : (mc + 1) * P],
                    rdiag,
                )
            for nck in range(NCH):
                for mc in range(NCH):
                    tp = setup_ps.tile([P, P], F32, name="tp", tag="tp")
                    nc.tensor.transpose(
                        tp, a_sb[mc][:, nck * P : (nck + 1) * P], ident
                    )
                    nc.scalar.copy(adjT[nck][:, mc * P : (mc + 1) * P], tp)

        yps = ctx.enter_context(tc.tile_pool(name="yps", bufs=3, space="PSUM"))
        ops = ctx.enter_context(tc.tile_pool(name="ops", bufs=3, space="PSUM"))

        for blk in range(nblk):
            t0 = blk * tb
            # sliding-window prefetch: issue the load PF blocks ahead NOW,
            # before this block's store is emitted
            if blk + PF < nblk:
                prefetched.append(load_x(blk + PF))
            xt = prefetched[blk]
            ot = op.tile([P, NCH, tb, N_FEAT], BF16, name=f"o_{blk}", tag="o")
            # Phase 1: aggregation matmuls, 4 time steps per 2-bank PSUM
            # group, one ACT psum->sbuf bf16 copy per group. Back-to-back
            # GEMM1s keep PE busy while the copies land.
            ys_list = []
            for gi in range(ngrp):
                ypt2 = yps.tile([P, 2, N_NODES], F32, name="ypt2", tag="y")
                for q in range(2):
                    ti = gi * 2 + q
                    for ck in range(NCH):
                        nc.tensor.matmul(
                            ypt2[:, q, :],
                            xt[:, ck, ti, :],
                            adjT[ck],
                            start=(ck == 0),
                            stop=(ck == NCH - 1),
                        )
                ys2 = ysb.tile([P, 2, N_NODES], BF16, name=f"ys{gi}", tag="ys")
                nc.scalar.copy(ys2, ypt2)
                ys_list.append(ys2)
            # Phase 2: feature-transform matmuls into a (c, t4, o) PSUM
            # group, one DVE bias-add + bf16 cast per group
            for gi in range(ngrp):
                opt2 = ops.tile([P, NCH, 2, N_FEAT], F32, name="opt2", tag="op")
                for mc in range(NCH):
                    for q in range(2):
                        nc.tensor.matmul(
                            opt2[:, mc, q, :],
                            ys_list[gi][:, q, mc * P : (mc + 1) * P],
                            w_sb,
                            start=True,
                            stop=True,
                        )
                tt0 = gi * 2
                nc.vector.tensor_add(
                    ot[:, :, tt0 : tt0 + 2, :], opt2, bias_bc
                )
            store_eng(blk).dma_start(out=out4[:, :, t0 : t0 + tb, :], in_=ot)


def build(t_sh=T_SH, tb=16):
    """Build + compile the per-core Bass module."""
    nc = bacc.Bacc(
        "TRN2", target_bir_lowering=False, debug=False, num_devices=N_CORES
    )
    x = nc.dram_tensor("node_feats", [N_NODES, t_sh, N_FEAT], BF16, kind="ExternalInput").ap()
    adj = nc.dram_tensor("adj_matrix", [N_NODES, N_NODES], F32, kind="ExternalInput").ap()
    w = nc.dram_tensor("weight", [N_FEAT, N_FEAT], BF16, kind="ExternalInput").ap()
    b = nc.dram_tensor("bias", [N_FEAT], F32, kind="ExternalInput").ap()
    out = nc.dram_tensor("out", [N_NODES, t_sh, N_FEAT], BF16, kind="ExternalOutput").ap()
    with tile.TileContext(nc) as tc:
        _gcn_body(tc, out, x, adj, w, b, t_sh, tb)
    nc.compile()
    return nc


_built_nc = None


def _get_nc():
    global _built_nc
    if _built_nc is None:
        _built_nc = build()
    return _built_nc


def _run(node_feats, adj_matrix, weight, bias, trace=False, tmpdir=None):
    import ml_dtypes

    nc = _get_nc()
    node_feats = np.ascontiguousarray(node_feats, dtype=np.float32)
    adj_matrix = np.ascontiguousarray(adj_matrix, dtype=np.float32)
    weight = np.ascontiguousarray(weight, dtype=np.float32).astype(
        ml_dtypes.bfloat16
    )
    bias = np.ascontiguousarray(bias, dtype=np.float32)
    in_maps = [
        {
            "node_feats": np.ascontiguousarray(
                node_feats[:, c * T_SH : (c + 1) * T_SH, :]
            ).astype(ml_dtypes.bfloat16),
            "adj_matrix": adj_matrix,
            "weight": weight,
            "bias": bias,
        }
        for c in range(N_CORES)
    ]
    res = run_bass_kernel_spmd(
        nc, in_maps, list(range(N_CORES)), trace=trace, tmpdir=tmpdir
    )
    out = np.concatenate(
        [res.results[c]["out"] for c in range(N_CORES)], axis=1
    ).astype(np.float32)
    return out, res


def kernel(node_feats, adj_matrix, weight, bias):
    out, _ = _run(node_feats, adj_matrix, weight, bias)
    return out


# revision 19
# speedup vs baseline: 1.0997x; 1.0997x over previous
"""GCN layer kernel for Trainium2, SPMD over 8 NeuronCores.

Reference computation (all fp32):
    adj_hat = rownorm(adj + I)                      # [N, N]
    out     = adj_hat @ (X @ W) + bias              # X: [N, T, A]

Sharding: T (time) axis split across 8 cores; adj/W/bias replicated.

bf16 I/O: the correctness gate is rel_err < 2e-2 and the full-bf16
datapath measures 4e-3, so X and out travel as bf16 - HBM traffic per
core drops 67MB -> 33.5MB (DMA was 91% busy at fp32). bf16 also makes
every matmul 1 cyc/col at any width and enables FWL weight loads that
hide LDWEIGHTS under the previous matmul.

Node indices are PARITY-chunked (chunk c holds nodes {2i+c}) rather
than half-chunked, so the adjacency loads as ONE contiguous 256KB DMA
([128 part, 2KB]: partition p <- rows 2p, 2p+1). The natural-layout
alternative ([128, 1KB-strided] x2) is descriptor-bound (~16GB/s) and
kept the whole setup chain - and therefore the first matmul - waiting
~20us. The bias broadcast tile is built on-chip with a rank-1 matmul
for the same reason.

Per-core kernel (T_SH = 256 time steps, time blocks of tb=16):
  setup (once): adj in; r[m] = 1/(1+rowsum); fold the row normalization
    INTO the adjacency ((adj+I)*r) while m is still on partitions; 4 PE
    transposes of parity blocks -> adjT_hat[n, m] bf16 with m-cols
    ordered (j, p) to match the output layout. Epilogue then needs no
    per-partition scale.
  per pair of time steps (2 t per PSUM bank, amortizes copy fixed cost):
    G1: ypt2[a, (t2 m)] = matmul(lhsT=X_t[n,a] bf16, rhs=adjT_hat[n,m])
        x2 parity chunks x2 t -> one [128,512] PSUM bank
    ys2 = bf16(ypt2)                          (one ACT copy per 2 t)
    G2: ops2[m, (c t2 o)] = matmul(lhsT=ys2[a, m-chunk], rhs=W[a,o])
        x2 chunks x2 t -> one [128,512] PSUM bank
    out = bf16(ops2 + bias)                   (one DVE add per 2 t)
  Each HWDGE ring tops out ~200-300GB/s, so X loads alternate between
  the sync and scalar rings and stores ride gpsimd; the last blocks'
  stores fan out across rings (all loads are emitted by then, so no
  head-of-line blocking) and the final store is split in half across
  two rings to shorten the drain tail.
Host: converts X/W to bf16, slices T, and upcasts the bf16 output back
to fp32.
"""

import os
import sys

import numpy as np

for _p in ("/opt/trn_rl_repo", "/root/.axon_site/_ro/trn_rl_repo"):
    if os.path.isdir(_p) and _p not in sys.path:
        sys.path.insert(0, _p)

import concourse.bass as bass
import concourse.mybir as mybir
import concourse.tile as tile
from concourse import bacc
from concourse.bass_utils import run_bass_kernel_spmd
from concourse.masks import make_identity

N_NODES = 256
N_TIMES = 2048
N_FEAT = 128
N_CORES = 8
T_SH = N_TIMES // N_CORES  # 256 time steps per core
P = 128  # partitions
NCH = N_NODES // P  # 2 node parity chunks

F32 = mybir.dt.float32
BF16 = mybir.dt.bfloat16


def _gcn_body(tc, out, x, adj, w, b, t_sh, tb):
    nc = tc.nc
    nblk = t_sh // tb
    ngrp = tb // 2  # 2 time steps per PSUM bank

    from contextlib import ExitStack

    with ExitStack() as ctx:
        const = ctx.enter_context(tc.tile_pool(name="const", bufs=1))
        setup = ctx.enter_context(tc.tile_pool(name="setup", bufs=1))

        # --- setup DMAs first, on the gpsimd ring (it carries no bulk
        # traffic until the first store ~25us in), so the adjacency is
        # resident long before the X prefetch could have drained. adj is
        # ONE contiguous 256KB transfer: partition p <- rows 2p, 2p+1.
        a_sb = setup.tile([P, 2, N_NODES], F32, name="a", tag="a")
        nc.gpsimd.dma_start(out=a_sb, in_=adj.rearrange("(p j) n -> p j n", j=2))

        w_sb = const.tile([P, N_FEAT], BF16)
        nc.gpsimd.dma_start(out=w_sb, in_=w)

        # bias lands as a single [1, 128] row; the partition broadcast is
        # done on-chip (rank-1 matmul) - a [0,128]-broadcast DMA would be
        # descriptor-bound and hog the queue for ~10us.
        bias_row = setup.tile([1, N_FEAT], F32, name="brow", tag="brow")
        bias_row_ap = bass.AP(
            tensor=b.tensor, offset=b.offset, ap=[[0, 1], b.ap[0]]
        )
        nc.gpsimd.dma_start(out=bias_row, in_=bias_row_ap)

        ident = const.tile([P, P], F32)
        make_identity(nc, ident)

        # bias replicated across partitions and duplicated (c, t2) so one
        # DVE add per 2 time steps covers a whole [c, t2, o] PSUM bank
        bias_bc3 = const.tile([P, NCH * 2, N_FEAT], F32)
        bias_bc = bias_bc3.rearrange("p (c q) o -> p c q o", c=NCH)

        # adjT_hat[n, m] = (adj[m, n] + I) / deg[m]: n on partitions in
        # parity order (partition q of chunk k <-> node 2q+k), m in the
        # free dim ordered (j, p) <-> node 2p+j.
        adjT = [
            const.tile([P, N_NODES], BF16, name=f"adjT{c}", tag=f"adjT{c}")
            for c in range(NCH)
        ]

        # Main-loop SBUF pools are created BEFORE the setup scratch pool's
        # remaining tiles so their addresses don't alias the setup chain.
        xp = ctx.enter_context(tc.tile_pool(name="xp", bufs=6))
        op = ctx.enter_context(tc.tile_pool(name="op", bufs=3))
        ysb = ctx.enter_context(tc.tile_pool(name="ysb", bufs=ngrp + 2))

        # [n, t, a] viewed as [n//2, n%2, t, a]: parity chunking, one 1MB
        # DMA moves both parity chunks of a time block (4KB runs)
        x4 = x.rearrange("(n c) t a -> n c t a", c=NCH)
        out4 = out.rearrange("(m c) t a -> m c t a", c=NCH)

        # Each HWDGE ring tops out around ~200GB/s. Directions stay
        # DISJOINT per ring (a store descriptor waiting on its epilogue
        # would head-of-line block later loads in the same FIFO): loads
        # alternate sync/scalar, stores go to gpsimd except the tail.
        load_eng = [nc.sync, nc.scalar]

        def load_x(blk):
            t0 = blk * tb
            xtc = xp.tile([P, NCH, tb, N_FEAT], BF16, name=f"x_{blk}", tag="x")
            load_eng[blk % 2].dma_start(out=xtc, in_=x4[:, :, t0 : t0 + tb, :])
            return xtc

        PF = 4  # prefetch depth (< xp bufs)
        prefetched = [load_x(blk) for blk in range(min(PF, nblk))]

        with tc.tile_pool(name="setup_ps", bufs=1, space="PSUM") as setup_ps:
            # broadcast bias across partitions: psum[i, o] = bias[o]
            ones1 = setup.tile([1, P], F32, name="ones1", tag="ones1")
            nc.vector.memset(ones1, 1.0)
            bps = setup_ps.tile([P, N_FEAT], F32, name="bps", tag="bps")
            nc.tensor.matmul(bps, ones1, bias_row, start=True, stop=True)
            for rep in range(NCH * 2):
                nc.scalar.copy(bias_bc3[:, rep, :], bps)

            # r_j[p] = 1 / (1 + sum_n adj[2p+j, n]); fold the row
            # normalization in BEFORE the transpose, while the row index
            # m=2p+j is still (partition, slice)
            r = []
            for j in range(NCH):
                dg = setup.tile([P, 1], F32, name=f"dg{j}", tag=f"dg{j}")
                nc.vector.reduce_sum(dg, a_sb[:, j, :], axis=mybir.AxisListType.X)
                nc.vector.tensor_scalar_add(dg, dg, 1.0)
                rj = setup.tile([P, 1], F32, name=f"r{j}", tag=f"r{j}")
                nc.vector.reciprocal(rj, dg)
                nc.vector.tensor_scalar_mul(a_sb[:, j, :], a_sb[:, j, :], rj)
                r.append(rj)

            # diagonal of (adj + I) * r: node m=2p+j is column n=2p+j,
            # which sits in parity block (j, k=j) at (p, p)
            a_blk = a_sb.rearrange("p j (q k) -> p j q k", k=NCH)
            for j in range(NCH):
                rdiag = setup.tile([P, P], F32, name=f"rd{j}", tag=f"rd{j}")
                nc.vector.tensor_scalar_mul(rdiag, ident, r[j])
                nc.vector.tensor_add(
                    a_blk[:, j, :, j], a_blk[:, j, :, j], rdiag
                )

            # transpose parity blocks: B_jk[p, q] = a_hat[2p+j, 2q+k]
            # -> adjT[k][q, j*128+p]
            for k in range(NCH):
                for j in range(NCH):
                    tp = setup_ps.tile([P, P], F32, name="tp", tag="tp")
                    nc.tensor.transpose(tp, a_blk[:, j, :, k], ident)
                    nc.scalar.copy(adjT[k][:, j * P : (j + 1) * P], tp)

        yps = ctx.enter_context(tc.tile_pool(name="yps", bufs=3, space="PSUM"))
        ops = ctx.enter_context(tc.tile_pool(name="ops", bufs=3, space="PSUM"))

        for blk in range(nblk):
            t0 = blk * tb
            # sliding-window prefetch: issue the load PF blocks ahead NOW,
            # before this block's store is emitted
            if blk + PF < nblk:
                prefetched.append(load_x(blk + PF))
            xt = prefetched[blk]
            ot = op.tile([P, NCH, tb, N_FEAT], BF16, name=f"o_{blk}", tag="o")
            # Phase 1: aggregation matmuls, 2 time steps per PSUM bank, one
            # ACT psum->sbuf bf16 copy per pair. Back-to-back GEMM1s keep
            # PE busy while the copies land.
            ys_list = []
            for gi in range(ngrp):
                ypt2 = yps.tile([P, 2, N_NODES], F32, name="ypt2", tag="y")
                for q in range(2):
                    ti = gi * 2 + q
                    for ck in range(NCH):
                        nc.tensor.matmul(
                            ypt2[:, q, :],
                            xt[:, ck, ti, :],
                            adjT[ck],
                            start=(ck == 0),
                            stop=(ck == NCH - 1),
                        )
                ys2 = ysb.tile([P, 2, N_NODES], BF16, name=f"ys{gi}", tag="ys")
                nc.scalar.copy(ys2, ypt2)
                ys_list.append(ys2)
            # Phase 2: feature-transform matmuls into a (c, t2, o) PSUM
            # bank, one DVE bias-add + bf16 cast per pair
            for gi in range(ngrp):
                opt2 = ops.tile([P, NCH, 2, N_FEAT], F32, name="opt2", tag="op")
                for mc in range(NCH):
                    for q in range(2):
                        nc.tensor.matmul(
                            opt2[:, mc, q, :],
                            ys_list[gi][:, q, mc * P : (mc + 1) * P],
                            w_sb,
                            start=True,
                            stop=True,
                        )
                tt0 = gi * 2
                nc.vector.tensor_add(
                    ot[:, :, tt0 : tt0 + 2, :], opt2, bias_bc
                )
            # stores: gpsimd until the tail; the very last block is split
            # in half across sync+scalar so the drain overlaps
            if blk == nblk - 1 and tb >= 2:
                h = tb // 2
                nc.sync.dma_start(
                    out=out4[:, :, t0 : t0 + h, :], in_=ot[:, :, 0:h, :]
                )
                nc.scalar.dma_start(
                    out=out4[:, :, t0 + h : t0 + tb, :], in_=ot[:, :, h:tb, :]
                )
            elif blk >= nblk - 4:
                eng = [nc.sync, nc.scalar, nc.gpsimd][blk % 3]
                eng.dma_start(out=out4[:, :, t0 : t0 + tb, :], in_=ot)
            else:
                nc.gpsimd.dma_start(out=out4[:, :, t0 : t0 + tb, :], in_=ot)


def build(t_sh=T_SH, tb=16):
    """Build + compile the per-core Bass module."""
    nc = bacc.Bacc(
        "TRN2", target_bir_lowering=False, debug=False, num_devices=N_CORES
    )
    x = nc.dram_tensor("node_feats", [N_NODES, t_sh, N_FEAT], BF16, kind="ExternalInput").ap()
    adj = nc.dram_tensor("adj_matrix", [N_NODES, N_NODES], F32, kind="ExternalInput").ap()
    w = nc.dram_tensor("weight", [N_FEAT, N_FEAT], BF16, kind="ExternalInput").ap()
    b = nc.dram_tensor("bias", [N_FEAT], F32, kind="ExternalInput").ap()
    out = nc.dram_tensor("out", [N_NODES, t_sh, N_FEAT], BF16, kind="ExternalOutput").ap()
    with tile.TileContext(nc) as tc:
        _gcn_body(tc, out, x, adj, w, b, t_sh, tb)
    nc.compile()
    return nc


_built_nc = None


def _get_nc():
    global _built_nc
    if _built_nc is None:
        _built_nc = build()
    return _built_nc


def _run(node_feats, adj_matrix, weight, bias, trace=False, tmpdir=None):
    import ml_dtypes

    nc = _get_nc()
    node_feats = np.ascontiguousarray(node_feats, dtype=np.float32)
    adj_matrix = np.ascontiguousarray(adj_matrix, dtype=np.float32)
    weight = np.ascontiguousarray(weight, dtype=np.float32).astype(
        ml_dtypes.bfloat16
    )
    bias = np.ascontiguousarray(bias, dtype=np.float32)
    in_maps = [
        {
            "node_feats": np.ascontiguousarray(
                node_feats[:, c * T_SH : (c + 1) * T_SH, :]
            ).astype(ml_dtypes.bfloat16),
            "adj_matrix": adj_matrix,
            "weight": weight,
            "bias": bias,
        }
        for c in range(N_CORES)
    ]
    res = run_bass_kernel_spmd(
        nc, in_maps, list(range(N_CORES)), trace=trace, tmpdir=tmpdir
    )
    out = np.concatenate(
        [res.results[c]["out"] for c in range(N_CORES)], axis=1
    ).astype(np.float32)
    return out, res


def kernel(node_feats, adj_matrix, weight, bias):
    out, _ = _run(node_feats, adj_matrix, weight, bias)
    return out


# revision 22
# speedup vs baseline: 1.1342x; 1.0314x over previous
"""GCN layer kernel for Trainium2, SPMD over 8 NeuronCores.

Reference computation (all fp32):
    adj_hat = rownorm(adj + I)                      # [N, N]
    out     = adj_hat @ (X @ W) + bias              # X: [N, T, A]

Sharding: T (time) axis split across 8 cores; adj/W/bias replicated.

bf16 I/O: the correctness gate is rel_err < 2e-2 and the full-bf16
datapath measures 4e-3, so X and out travel as bf16 - HBM traffic per
core drops 67MB -> 33.5MB (DMA was 91% busy at fp32). bf16 also makes
every matmul 1 cyc/col at any width and enables FWL weight loads that
hide LDWEIGHTS under the previous matmul.

Node indices are PARITY-chunked (chunk c holds nodes {2i+c}) rather
than half-chunked, so the adjacency loads as ONE contiguous 256KB DMA
([128 part, 2KB]: partition p <- rows 2p, 2p+1). The natural-layout
alternative ([128, 1KB-strided] x2) is descriptor-bound (~16GB/s) and
kept the whole setup chain - and therefore the first matmul - waiting
~20us. The bias broadcast tile is built on-chip with a rank-1 matmul
for the same reason.

Per-core kernel (T_SH = 256 time steps, time blocks of tb=16):
  setup (once): adj in; r[m] = 1/(1+rowsum); fold the row normalization
    INTO the adjacency ((adj+I)*r) while m is still on partitions; 4 PE
    transposes of parity blocks -> adjT_hat[n, m] bf16 with m-cols
    ordered (j, p) to match the output layout. Epilogue then needs no
    per-partition scale.
  per pair of time steps (2 t per PSUM bank, amortizes copy fixed cost):
    G1: ypt2[a, (t2 m)] = matmul(lhsT=X_t[n,a] bf16, rhs=adjT_hat[n,m])
        x2 parity chunks x2 t -> one [128,512] PSUM bank
    ys2 = bf16(ypt2)                          (one ACT copy per 2 t)
    G2: ops2[m, (c t2 o)] = matmul(lhsT=ys2[a, m-chunk], rhs=W[a,o])
        x2 chunks x2 t -> one [128,512] PSUM bank
    out = bf16(ops2 + bias)                   (one DVE add per 2 t)
  Each HWDGE ring tops out ~200-300GB/s, so X loads alternate between
  the sync and scalar rings and stores ride gpsimd; the last blocks'
  stores fan out across rings (all loads are emitted by then, so no
  head-of-line blocking) and the final store is split in half across
  two rings to shorten the drain tail.
Host: converts X/W to bf16, slices T, and upcasts the bf16 output back
to fp32.
"""

import os
import sys

import numpy as np

for _p in ("/opt/trn_rl_repo", "/root/.axon_site/_ro/trn_rl_repo"):
    if os.path.isdir(_p) and _p not in sys.path:
        sys.path.insert(0, _p)

import concourse.bass as bass
import concourse.mybir as mybir
import concourse.tile as tile
from concourse import bacc
from concourse.bass_utils import run_bass_kernel_spmd
from concourse.masks import make_identity

N_NODES = 256
N_TIMES = 2048
N_FEAT = 128
N_CORES = 8
T_SH = N_TIMES // N_CORES  # 256 time steps per core
P = 128  # partitions
NCH = N_NODES // P  # 2 node parity chunks

F32 = mybir.dt.float32
BF16 = mybir.dt.bfloat16


def _gcn_body(tc, out, x, adj, w, b, t_sh, tb):
    nc = tc.nc
    nblk = t_sh // tb
    ngrp = tb // 2  # 2 time steps per PSUM bank

    from contextlib import ExitStack

    with ExitStack() as ctx:
        const = ctx.enter_context(tc.tile_pool(name="const", bufs=1))
        setup = ctx.enter_context(tc.tile_pool(name="setup", bufs=1))

        # --- setup DMAs first, at the HEAD of the two hardware rings
        # (sync/scalar), ahead of the X prefetch. The gpsimd "ring" is a
        # software-dynamic queue whose Q7-generated descriptors only start
        # flowing ~12us in - too late for the setup chain. adj is ONE
        # contiguous 256KB transfer: partition p <- rows 2p, 2p+1.
        a_sb = setup.tile([P, 2, N_NODES], F32, name="a", tag="a")
        nc.sync.dma_start(out=a_sb, in_=adj.rearrange("(p j) n -> p j n", j=2))

        w_sb = const.tile([P, N_FEAT], BF16)
        nc.scalar.dma_start(out=w_sb, in_=w)

        # bias lands as a single [1, 128] row; the partition broadcast is
        # done on-chip (rank-1 matmul) - a [0,128]-broadcast DMA would be
        # descriptor-bound and hog the queue for ~10us.
        bias_row = setup.tile([1, N_FEAT], F32, name="brow", tag="brow")
        bias_row_ap = bass.AP(
            tensor=b.tensor, offset=b.offset, ap=[[0, 1], b.ap[0]]
        )
        nc.scalar.dma_start(out=bias_row, in_=bias_row_ap)

        ident = const.tile([P, P], F32)
        make_identity(nc, ident)

        # bias replicated across partitions and duplicated (c, t2) so one
        # DVE add per 2 time steps covers a whole [c, t2, o] PSUM bank
        bias_bc3 = const.tile([P, NCH * 2, N_FEAT], F32)
        bias_bc = bias_bc3.rearrange("p (c q) o -> p c q o", c=NCH)

        # adjT_hat[n, m] = (adj[m, n] + I) / deg[m]: n on partitions in
        # parity order (partition q of chunk k <-> node 2q+k), m in the
        # free dim ordered (j, p) <-> node 2p+j.
        adjT = [
            const.tile([P, N_NODES], BF16, name=f"adjT{c}", tag=f"adjT{c}")
            for c in range(NCH)
        ]

        # Main-loop SBUF pools are created BEFORE the setup scratch pool's
        # remaining tiles so their addresses don't alias the setup chain.
        xp = ctx.enter_context(tc.tile_pool(name="xp", bufs=6))
        op = ctx.enter_context(tc.tile_pool(name="op", bufs=3))
        ysb = ctx.enter_context(tc.tile_pool(name="ysb", bufs=ngrp + 2))

        # [n, t, a] viewed as [n//2, n%2, t, a]: parity chunking, one 1MB
        # DMA moves both parity chunks of a time block (4KB runs)
        x4 = x.rearrange("(n c) t a -> n c t a", c=NCH)
        out4 = out.rearrange("(m c) t a -> m c t a", c=NCH)

        # Each HWDGE ring tops out around ~200GB/s. Directions stay
        # DISJOINT per ring (a store descriptor waiting on its epilogue
        # would head-of-line block later loads in the same FIFO): loads
        # alternate sync/scalar, stores go to gpsimd except the tail.
        load_eng = [nc.sync, nc.scalar]

        def load_x(blk):
            t0 = blk * tb
            xtc = xp.tile([P, NCH, tb, N_FEAT], BF16, name=f"x_{blk}", tag="x")
            if blk == 0 and tb >= 2:
                # block 0 gates the first matmul: split it across both
                # rings so it lands ~2x sooner
                h = tb // 2
                nc.sync.dma_start(
                    out=xtc[:, :, 0:h, :], in_=x4[:, :, t0 : t0 + h, :]
                )
                nc.scalar.dma_start(
                    out=xtc[:, :, h:tb, :], in_=x4[:, :, t0 + h : t0 + tb, :]
                )
            else:
                load_eng[blk % 2].dma_start(
                    out=xtc, in_=x4[:, :, t0 : t0 + tb, :]
                )
            return xtc

        PF = 4  # prefetch depth (< xp bufs)
        prefetched = [load_x(blk) for blk in range(min(PF, nblk))]

        with tc.tile_pool(name="setup_ps", bufs=1, space="PSUM") as setup_ps:
            # broadcast bias across partitions: psum[i, o] = bias[o]
            ones1 = setup.tile([1, P], F32, name="ones1", tag="ones1")
            nc.vector.memset(ones1, 1.0)
            bps = setup_ps.tile([P, N_FEAT], F32, name="bps", tag="bps")
            nc.tensor.matmul(bps, ones1, bias_row, start=True, stop=True)
            # bias replication on DVE - keeps the ACT queue free for the
            # adjT copies and the first ys copies
            for rep in range(NCH * 2):
                nc.vector.tensor_copy(bias_bc3[:, rep, :], bps)

            # r_j[p] = 1 / (1 + sum_n adj[2p+j, n]); fold the row
            # normalization in BEFORE the transpose, while the row index
            # m=2p+j is still (partition, slice)
            r = []
            for j in range(NCH):
                dg = setup.tile([P, 1], F32, name=f"dg{j}", tag=f"dg{j}")
                nc.vector.reduce_sum(dg, a_sb[:, j, :], axis=mybir.AxisListType.X)
                nc.vector.tensor_scalar_add(dg, dg, 1.0)
                rj = setup.tile([P, 1], F32, name=f"r{j}", tag=f"r{j}")
                nc.vector.reciprocal(rj, dg)
                nc.vector.tensor_scalar_mul(a_sb[:, j, :], a_sb[:, j, :], rj)
                r.append(rj)

            # diagonal of (adj + I) * r: node m=2p+j is column n=2p+j,
            # which sits in parity block (j, k=j) at (p, p)
            a_blk = a_sb.rearrange("p j (q k) -> p j q k", k=NCH)
            for j in range(NCH):
                rdiag = setup.tile([P, P], F32, name=f"rd{j}", tag=f"rd{j}")
                nc.vector.tensor_scalar_mul(rdiag, ident, r[j])
                nc.vector.tensor_add(
                    a_blk[:, j, :, j], a_blk[:, j, :, j], rdiag
                )

            # transpose parity blocks: B_jk[p, q] = a_hat[2p+j, 2q+k]
            # -> adjT[k][q, j*128+p]
            for k in range(NCH):
                for j in range(NCH):
                    tp = setup_ps.tile([P, P], F32, name="tp", tag="tp")
                    nc.tensor.transpose(tp, a_blk[:, j, :, k], ident)
                    nc.scalar.copy(adjT[k][:, j * P : (j + 1) * P], tp)

        yps = ctx.enter_context(tc.tile_pool(name="yps", bufs=3, space="PSUM"))
        ops = ctx.enter_context(tc.tile_pool(name="ops", bufs=3, space="PSUM"))

        for blk in range(nblk):
            t0 = blk * tb
            # sliding-window prefetch: issue the load PF blocks ahead NOW,
            # before this block's store is emitted
            if blk + PF < nblk:
                prefetched.append(load_x(blk + PF))
            xt = prefetched[blk]
            ot = op.tile([P, NCH, tb, N_FEAT], BF16, name=f"o_{blk}", tag="o")
            # Phase 1: aggregation matmuls, 2 time steps per PSUM bank, one
            # ACT psum->sbuf bf16 copy per pair. Back-to-back GEMM1s keep
            # PE busy while the copies land.
            ys_list = []
            for gi in range(ngrp):
                ypt2 = yps.tile([P, 2, N_NODES], F32, name="ypt2", tag="y")
                for q in range(2):
                    ti = gi * 2 + q
                    for ck in range(NCH):
                        nc.tensor.matmul(
                            ypt2[:, q, :],
                            xt[:, ck, ti, :],
                            adjT[ck],
                            start=(ck == 0),
                            stop=(ck == NCH - 1),
                        )
                ys2 = ysb.tile([P, 2, N_NODES], BF16, name=f"ys{gi}", tag="ys")
                nc.scalar.copy(ys2, ypt2)
                ys_list.append(ys2)
            # Phase 2: feature-transform matmuls into a (c, t2, o) PSUM
            # bank, one DVE bias-add + bf16 cast per pair
            for gi in range(ngrp):
                opt2 = ops.tile([P, NCH, 2, N_FEAT], F32, name="opt2", tag="op")
                for mc in range(NCH):
                    for q in range(2):
                        nc.tensor.matmul(
                            opt2[:, mc, q, :],
                            ys_list[gi][:, q, mc * P : (mc + 1) * P],
                            w_sb,
                            start=True,
                            stop=True,
                        )
                tt0 = gi * 2
                nc.vector.tensor_add(
                    ot[:, :, tt0 : tt0 + 2, :], opt2, bias_bc
                )
            # stores: gpsimd until the tail; the very last block is split
            # in half across sync+scalar so the drain overlaps
            if blk == nblk - 1 and tb >= 2:
                h = tb // 2
                nc.sync.dma_start(
                    out=out4[:, :, t0 : t0 + h, :], in_=ot[:, :, 0:h, :]
                )
                nc.scalar.dma_start(
                    out=out4[:, :, t0 + h : t0 + tb, :], in_=ot[:, :, h:tb, :]
                )
            elif blk >= nblk - 4:
                eng = [nc.sync, nc.scalar, nc.gpsimd][blk % 3]
                eng.dma_start(out=out4[:, :, t0 : t0 + tb, :], in_=ot)
            else:
                nc.gpsimd.dma_start(out=out4[:, :, t0 : t0 + tb, :], in_=ot)


def build(t_sh=T_SH, tb=16):
    """Build + compile the per-core Bass module."""
    nc = bacc.Bacc(
        "TRN2", target_bir_lowering=False, debug=False, num_devices=N_CORES
    )
    x = nc.dram_tensor("node_feats", [N_NODES, t_sh, N_FEAT], BF16, kind="ExternalInput").ap()
    adj = nc.dram_tensor("adj_matrix", [N_NODES, N_NODES], F32, kind="ExternalInput").ap()
    w = nc.dram_tensor("weight", [N_FEAT, N_FEAT], BF16, kind="ExternalInput").ap()
    b = nc.dram_tensor("bias", [N_FEAT], F32, kind="ExternalInput").ap()
    out = nc.dram_tensor("out", [N_NODES, t_sh, N_FEAT], BF16, kind="ExternalOutput").ap()
    with tile.TileContext(nc) as tc:
        _gcn_body(tc, out, x, adj, w, b, t_sh, tb)
    nc.compile()
    return nc


_built_nc = None


def _get_nc():
    global _built_nc
    if _built_nc is None:
        _built_nc = build()
    return _built_nc


def _run(node_feats, adj_matrix, weight, bias, trace=False, tmpdir=None):
    import ml_dtypes

    nc = _get_nc()
    node_feats = np.ascontiguousarray(node_feats, dtype=np.float32)
    adj_matrix = np.ascontiguousarray(adj_matrix, dtype=np.float32)
    weight = np.ascontiguousarray(weight, dtype=np.float32).astype(
        ml_dtypes.bfloat16
    )
    bias = np.ascontiguousarray(bias, dtype=np.float32)
    in_maps = [
        {
            "node_feats": np.ascontiguousarray(
                node_feats[:, c * T_SH : (c + 1) * T_SH, :]
            ).astype(ml_dtypes.bfloat16),
            "adj_matrix": adj_matrix,
            "weight": weight,
            "bias": bias,
        }
        for c in range(N_CORES)
    ]
    res = run_bass_kernel_spmd(
        nc, in_maps, list(range(N_CORES)), trace=trace, tmpdir=tmpdir
    )
    out = np.concatenate(
        [res.results[c]["out"] for c in range(N_CORES)], axis=1
    ).astype(np.float32)
    return out, res


def kernel(node_feats, adj_matrix, weight, bias):
    out, _ = _run(node_feats, adj_matrix, weight, bias)
    return out
